# revision 62
# baseline (speedup 1.0000x reference)
"""Trainium2 Bass kernel for nn_Attention_5514738008849.

Dense transformer attention block with axial rotary embeddings:
  x:(8,1024,1024) -> qkv -> rope(q,k) -> softmax(qk^T/sqrt(d)) v -> proj+bias

Sharding: pure data-parallel over batch B=8 across the 8 NeuronCores (one
batch element per core, full weights replicated). No collectives.

v2 design: one fused software pipeline over 8 head-pair "slots" so the
Activation engine's ~133us of exp work hides completely under PE matmuls
(the v1 kernel ran attention as its own phase, ACT-bound at 93%):

  - all moving matmul operands bf16 (1 PE cycle/row with no sub-256
    free-size penalty; half the DMA bytes of f32); q/k kept f32->f32r for
    the logits matmul so rotary adds no quantization on the exp input
  - slot s computes QKV for pair s while running attention for pair s-1,
    interleaved per-kt so PE never out-runs the exp stream:
      A: lg(a,h0,kt) | avt(a-1,h1,qc=kt) | qk-q(p,kb=kt)
      B: lg(a,h1,kt) | avt(a,h0,qc=kt)   | qk-k(p,kb=kt)
      C: v(p,tb) region chains
  - rotary: DVE stream_shuffle does the interleaved pair-swap (the sin
    table carries the signs), so no PE shuffle matmul and no ACT copies
  - AV is computed transposed (out[q,d], V|ones as the 65-wide moving
    tensor): 27.7us of PE streams instead of 54.6; the softmax denominator
    rides in column 64; normalization is one reciprocal + one stride-0
    broadcast multiply per head on DVE
  - token-major -> c-major fixup for proj via xbar DMA transposes (on the
    otherwise-idle DMA path), prefetched w_proj, proj tail partially
    overlapped into the last attention slot
"""

import os
import sys

sys.path.insert(0, "/opt/trn_rl_repo")

# This kernel needs the axon-tunneled NeuronCores. A JAX_PLATFORMS=cpu pin
# (used by some harnesses for the jax reference) would prevent the axon
# backend from registering; clearing it here is a no-op when jax has already
# initialized and restores device visibility when it hasn't.
if os.environ.get("JAX_PLATFORMS", "") not in ("", None):
    if "axon" not in os.environ["JAX_PLATFORMS"]:
        os.environ.pop("JAX_PLATFORMS", None)

import ml_dtypes
import numpy as np

import concourse.bass as bass
import concourse.bacc as bacc_mod
import concourse.mybir as mybir
from concourse.bass_utils import run_bass_kernel_spmd
from concourse.tile import TileContext

B, N, C = 8, 1024, 1024
H, D = 16, 64          # heads, head dim
ROT = 32               # rotary dims per head (head_dim // 2)
FH = FW = 32           # token grid for axial rope
NP = 8                 # head pairs (= weight column blocks of 128)
NCORES = 8
F32 = mybir.dt.float32
F32R = mybir.dt.float32r
BF16 = mybir.dt.bfloat16
NPBF16 = ml_dtypes.bfloat16
EXP = mybir.ActivationFunctionType.Exp

# out[i] = in[i^1] within each 32-partition group: the rotate_half pair swap
SWAP_MASK = [i ^ 1 for i in range(32)]


def _host_tables():
    """Rotary cos / signed-sin in d-major (dim-on-partition) layout.

    Rows of a 128-partition q/k block: [0:32] rot dims head-even,
    [32:64] pass head-even, [64:96] rot head-odd, [96:128] pass head-odd.
    The sin table is 0 on pass rows and carries the rotate_half signs
    (-sin on even rot rows, +sin on odd) so that
      q_rot = q * cos + swap(q) * sin'
    """
    dim_r = D // 4                                    # 16
    base = np.linspace(1.0, (FH * FW) / 2.0, dim_r // 2) * np.pi   # (8,)

    def axis_freqs(n):
        pos = np.linspace(-1.0, 1.0, n)
        f = pos[:, None] * base[None, :]              # (n, 8)
        return np.repeat(f, 2, axis=-1)               # (n, 16)

    fH = np.broadcast_to(axis_freqs(FH)[:, None, :], (FH, FW, dim_r))
    fW = np.broadcast_to(axis_freqs(FW)[None, :, :], (FH, FW, dim_r))
    freqs = np.concatenate([fH, fW], axis=-1).reshape(N, ROT)      # (1024, 32)

    cos_d = np.ones((128, N), np.float32)
    sin_d = np.zeros((128, N), np.float32)
    ct = np.cos(freqs).T.astype(np.float32)           # (32, 1024)
    st = np.sin(freqs).T.astype(np.float32)
    sgn = np.where(np.arange(ROT) % 2 == 0, -1.0, 1.0)[:, None].astype(np.float32)
    sp = st * sgn
    cos_d[0:32] = ct
    cos_d[64:96] = ct
    sin_d[0:32] = sp
    sin_d[64:96] = sp
    return cos_d, sin_d


DEBUG_TAPS = False


def _build_program():
    nc = bacc_mod.Bacc()
    xt_h = nc.declare_dram_parameter("xt", [C, N], BF16, isOutput=False)
    # per-pair packed [q|k|v] weight columns: rows p*1024+kb*128, cols 384
    wqkv_h = nc.declare_dram_parameter("wqkv", [NP * C, 384], BF16, isOutput=False)
    wproj_h = nc.declare_dram_parameter("w_proj", [C, C], BF16, isOutput=False)
    brow_h = nc.declare_dram_parameter("b_row", [1, C], BF16, isOutput=False)
    cos_h = nc.declare_dram_parameter("cos_d", [128, N], BF16, isOutput=False)
    sin_h = nc.declare_dram_parameter("sin_d", [128, N], BF16, isOutput=False)
    ones_h = nc.declare_dram_parameter("ones_row", [1, 128], BF16, isOutput=False)
    out_h = nc.declare_dram_parameter("out", [N, C], BF16, isOutput=True)
    if DEBUG_TAPS:
        dbg_qrot = nc.declare_dram_parameter("dbg_qrot", [128, N], F32, isOutput=True)
        dbg_vext = nc.declare_dram_parameter("dbg_vext", [128, 8 * 2 * 65], BF16, isOutput=True)
        dbg_e = nc.declare_dram_parameter("dbg_e", [128, N], BF16, isOutput=True)
        dbg_ar = nc.declare_dram_parameter("dbg_ar", [128, 8 * 65], F32, isOutput=True)
        dbg_avn = nc.declare_dram_parameter("dbg_avn", [128, 8 * C], BF16, isOutput=True)
        dbg_at = nc.declare_dram_parameter("dbg_at", [128, 8 * C], BF16, isOutput=True)

    def f32r(ap):
        return ap.bitcast(F32R)

    with nc.allow_low_precision(reason="bf16 operands, fp32 accumulate"), \
         TileContext(nc) as tc, \
         tc.tile_pool(name="consts", bufs=1) as consts, \
         tc.tile_pool(name="persist", bufs=1) as persist, \
         tc.tile_pool(name="xtp", bufs=1) as xtp, \
         tc.tile_pool(name="wq", bufs=24) as wqp, \
         tc.tile_pool(name="rotp", bufs=2) as rotp, \
         tc.tile_pool(name="vextp", bufs=2) as vextp, \
         tc.tile_pool(name="ep", bufs=24) as ep, \
         tc.tile_pool(name="avrawp", bufs=4) as avrawp, \
         tc.tile_pool(name="recipp", bufs=4) as recipp, \
         tc.tile_pool(name="youtp", bufs=2) as youtp:

        cos_sb = consts.tile([128, N], BF16)
        sin_sb = consts.tile([128, N], BF16)
        ones_sb = consts.tile([1, 128], BF16)
        brow_sb = consts.tile([1, C], BF16)
        wp_sb = persist.tile([128, 8, C], BF16)    # w_proj rows-on-partitions
        avn_sb = persist.tile([128, 8, C], BF16)   # normalized AV, token-major
        attnT_sb = persist.tile([128, 8, C], BF16)  # AV^T, c-major (for proj)
        xT_sb = xtp.tile([128, 8, N], BF16)

        wt = {}          # pair -> list of 8 [128, 384] weight tiles
        qrot, krot = {}, {}
        vext = {}
        etiles = {}
        avraw = {}

        def dma_pair_w(p):
            tiles = []
            for kb in range(8):
                t = wqp.tile([128, 384], BF16, tag="w", name=f"w{p}_{kb}")
                nc.sync.dma_start(
                    out=t, in_=wqkv_h[p * C + kb * 128:p * C + (kb + 1) * 128, :]
                )
                tiles.append(t)
            wt[p] = tiles

        # ---- initial loads: pair 0/1 weights on HWDGE, x^T on SWDGE.
        # DMA_ENGINES is a serialized resource in the timeline model, so
        # keep the startup byte count minimal (w_proj waits until slot 2)
        # and issue w0[kb] / xT[kb] alternately so the first q chain's
        # step kb has both operands as early as possible.
        # x^T rides the (startup-idle) Activation HWDGE queue: no SWDGE
        # descriptor-generation latency, and it frees the Pool sequencer.
        w0_tiles = []
        for kb in range(8):
            t = wqp.tile([128, 384], BF16, tag="w", name=f"w0_{kb}")
            nc.sync.dma_start(
                out=t, in_=wqkv_h[kb * 128:(kb + 1) * 128, :]
            )
            w0_tiles.append(t)
            nc.scalar.dma_start(
                out=xT_sb[:, kb, :], in_=xt_h[kb * 128:(kb + 1) * 128, :]
            )
        wt[0] = w0_tiles
        nc.sync.dma_start(out=cos_sb, in_=cos_h[:, :])
        nc.sync.dma_start(out=sin_sb, in_=sin_h[:, :])
        nc.sync.dma_start(out=ones_sb, in_=ones_h[:, :])
        nc.sync.dma_start(out=brow_sb, in_=brow_h[:, :])
        dma_pair_w(1)

        def emit_qk_step(p, kb, which, ps):
            w = wt[p][kb][:, which * 128:(which + 1) * 128]
            for hf in range(2):
                nc.tensor.matmul(
                    ps[:, hf * 512:(hf + 1) * 512],
                    w,
                    xT_sb[:, kb, hf * 512:(hf + 1) * 512],
                    start=(kb == 0),
                    stop=(kb == 7),
                )

        def rot_ops(p, which, ps):
            """pair p's q (which=0) or k (which=1) rotary as a list of DVE
            op thunks: psum evacuation, partition pair-swap via
            stream_shuffle, and the cos/sin' combine, half 0 first (the
            next logits tiles only need columns 0:512). Returned thunks are
            emitted one-per-iteration inside the phase loops so short
            evacuations are not stuck behind them in DVE program order."""
            qs = rotp.tile([128, N], F32, tag="qsb", name=f"qs{p}_{which}")
            sh = rotp.tile([128, N], F32, tag="shuf", name=f"sh{p}_{which}")
            tmp = rotp.tile([128, N], F32, tag="tmp", name=f"tm{p}_{which}")
            dst = rotp.tile([128, N], F32, tag=("qrot" if which == 0 else "krot"),
                            name=f"rot{p}_{which}")
            (qrot if which == 0 else krot)[p] = dst
            ops = [
                lambda: nc.vector.tensor_copy(qs[:, 0:512], ps[:, 0:512]),
                lambda: nc.vector.tensor_copy(qs[:, 512:1024], ps[:, 512:1024]),
            ]
            for hf in range(2):
                c = slice(hf * 512, (hf + 1) * 512)
                # every write of dst rounds to fp32r: the logits matmul
                # consumes it as f32r and the verifier checks all writers
                ops += [
                    lambda c=c: nc.vector.stream_shuffle(sh[:, c], qs[:, c],
                                                         SWAP_MASK),
                    # all-SBUF multiply: runs on the idle Pool engine
                    lambda c=c: nc.gpsimd.tensor_mul(tmp[:, c], sh[:, c],
                                                     sin_sb[:, c]),
                    lambda c=c: nc.vector.tensor_mul(f32r(dst[:, c]), qs[:, c],
                                                     cos_sb[:, c]),
                    lambda c=c: nc.vector.tensor_add(f32r(dst[:, c]),
                                                     dst[:, c], tmp[:, c]),
                ]
            return ops

        def emit_rot(p, which, ps):
            for op in rot_ops(p, which, ps):
                op()

        def new_vext(p):
            vt = vextp.tile([128, 8, 2, 65], BF16, tag="vext", name=f"vext{p}")
            nc.gpsimd.memset(vt[:, :, :, 64:65], 1.0)
            vext[p] = vt

        def emit_v_chain(p, tb, ps, evac=True):
            for kb in range(8):
                nc.tensor.matmul(
                    ps[:, tb * 128:(tb + 1) * 128],
                    xT_sb[:, kb, tb * 128:(tb + 1) * 128],
                    wt[p][kb][:, 256:384],
                    start=(kb == 0),
                    stop=(kb == 7),
                    skip_group_check=True,
                )
            if evac:
                # paired evacuation on ACT (idle during phase C): tb-1, tb
                nc.scalar.copy(
                    vext[p][:, tb - 1:tb + 1, :, 0:64],
                    ps[:, (tb - 1) * 128:(tb + 1) * 128].rearrange(
                        "p (t h d) -> p t h d", t=2, h=2
                    ),
                )

        def emit_proj_cb(qb, cb, y):
            for hf in range(2):
                nc.tensor.matmul(
                    y[:, hf * 512:(hf + 1) * 512],
                    attnT_sb[:, cb, qb * 128:(qb + 1) * 128],
                    wp_sb[:, cb, hf * 512:(hf + 1) * 512],
                    start=(cb == 0),
                    stop=False,
                )

        def emit_proj_bias(y):
            for hf in range(2):
                nc.tensor.matmul(
                    y[:, hf * 512:(hf + 1) * 512],
                    ones_sb,
                    brow_sb[:, hf * 512:(hf + 1) * 512],
                    start=False,
                    stop=True,
                )

        def finish_y(qb, y):
            # evacuate+DMA in halves so the output DMA overlaps the copy;
            # bf16 on the wire (host converts back to f32) halves the
            # serialized DMA time on the tail
            ysb = youtp.tile([128, C], BF16, tag="y", name=f"y{qb}")
            for hf in range(2):
                nc.scalar.copy(ysb[:, hf * 512:(hf + 1) * 512],
                               y[:, hf * 512:(hf + 1) * 512])
                nc.sync.dma_start(
                    out=out_h[qb * 128:(qb + 1) * 128, hf * 512:(hf + 1) * 512],
                    in_=ysb[:, hf * 512:(hf + 1) * 512],
                )

        def emit_norm(a, h):
            ar = avraw[(a, h)]
            rc = recipp.tile([128, 8, 1], F32, tag="rc", name=f"rc{a}_{h}")
            nc.vector.reciprocal(rc, ar[:, :, 64:65])
            gh = 2 * a + h
            # all-SBUF multiply -> runs on the otherwise-idle Pool engine
            nc.gpsimd.tensor_mul(
                avn_sb[:, :, gh * 64:(gh + 1) * 64],
                ar[:, :, 0:64],
                rc.broadcast_to([128, 8, 64]),
            )

        def emit_transpose(a):
            for qc in range(8):
                nc.sync.dma_start_transpose(
                    out=attnT_sb[:, a, qc * 128:(qc + 1) * 128],
                    in_=avn_sb[:, qc, a * 128:(a + 1) * 128],
                )

        with tc.tile_pool(name="ps_qkv", bufs=1, space="PSUM") as ps_qkv:
            y0 = None
            with tc.tile_pool(name="ps_lg", bufs=2, space="PSUM") as ps_lg, \
                 tc.tile_pool(name="ps_av", bufs=2, space="PSUM") as ps_av:

                def emit_lg_exp(a, h, kt):
                    lg = ps_lg.tile([128, N], F32, tag="lg", name=f"lg{a}_{h}_{kt}")
                    kr = krot[a][h * 64:(h + 1) * 64, kt * 128:(kt + 1) * 128]
                    for hf in range(2):
                        nc.tensor.matmul(
                            lg[:, hf * 512:(hf + 1) * 512],
                            f32r(kr),
                            f32r(qrot[a][h * 64:(h + 1) * 64,
                                         hf * 512:(hf + 1) * 512]),
                            start=True,
                            stop=True,
                        )
                    e = ep.tile([128, N], BF16, tag="e", name=f"e{a}_{h}_{kt}")
                    nc.scalar.activation(e, lg, EXP, scale=0.125)
                    etiles[(a, h, kt)] = e

                def emit_avt(a, h, j):
                    """AV chains for query chunks qc=2j and 2j+1 into one
                    psum tile: halves the tile-WAR boundaries and the DVE
                    evacuation count."""
                    av = ps_av.tile([128, 130], F32, tag="av", name=f"av{a}_{h}_{j}")
                    for qq in range(2):
                        qc = 2 * j + qq
                        for kt in range(8):
                            nc.tensor.matmul(
                                av[:, qq * 65:(qq + 1) * 65],
                                etiles[(a, h, kt)][:, qc * 128:(qc + 1) * 128],
                                vext[a][:, kt, h, :],
                                start=(kt == 0),
                                stop=(kt == 7),
                                skip_group_check=True,
                            )
                    if (a, h) not in avraw:
                        avraw[(a, h)] = avrawp.tile(
                            [128, 8, 65], F32, tag="ar", name=f"ar{a}_{h}"
                        )
                    nc.vector.tensor_copy(
                        avraw[(a, h)][:, 2 * j:2 * j + 2, :],
                        av.rearrange("p (q x) -> p q x", q=2),
                    )

                # ---- slot 0: QKV for pair 0 (DMA-paced; no attention).
                # k and v borrow the idle ps_lg tiles so no chain waits on
                # the previous chain's psum evacuation.
                q_ps = ps_qkv.tile([128, N], F32, tag="qkv", name="q0")
                for kb in range(8):
                    emit_qk_step(0, kb, 0, q_ps)
                emit_rot(0, 0, q_ps)
                if DEBUG_TAPS:
                    nc.sync.dma_start(out=dbg_qrot[:, :], in_=qrot[0])
                k_ps = ps_lg.tile([128, N], F32, tag="lg", name="k0")
                for kb in range(8):
                    emit_qk_step(0, kb, 1, k_ps)
                emit_rot(0, 1, k_ps)
                dma_pair_w(2)
                v_ps = ps_lg.tile([128, N], F32, tag="lg", name="v0")
                new_vext(0)
                for tb in range(8):
                    emit_v_chain(0, tb, v_ps, evac=(tb % 2 == 1))
                if DEBUG_TAPS:
                    nc.sync.dma_start(
                        out=dbg_vext[:, :],
                        in_=vext[0].rearrange("p a h x -> p (a h x)"),
                    )

                # ---- slots 1..8: attention for pair s-1 + QKV for pair s ----
                for s in range(1, 9):
                    a = s - 1
                    p = s if s < 8 else None
                    if p is not None and p + 1 < 8:
                        dma_pair_w(p + 1)
                    if s == 2:
                        for cb in range(8):
                            nc.sync.dma_start(
                                out=wp_sb[:, cb, :],
                                in_=wproj_h[cb * 128:(cb + 1) * 128, :],
                            )
                    if p is not None:
                        q_ps = ps_qkv.tile([128, N], F32, tag="qkv", name=f"q{p}")
                    else:
                        y0 = ps_qkv.tile([128, N], F32, tag="qkv", name="y0ps")
                    # phase A: lg(a,h0,kt>=2) | avt(a-1,h1) | qk-q(p)
                    # (kt 0/1 were prefetched in the previous slot's phase C;
                    # two avt chains held back as filler for the phase-B
                    # WAR wait on the q->k psum evacuation)
                    for kt in range(8):
                        if not (p is None and kt < 4):
                            emit_lg_exp(a, 0, kt)
                        if a >= 1 and kt in (0, 2, 4):
                            emit_avt(a - 1, 1, kt // 2)
                        if p is not None:
                            emit_qk_step(p, kt, 0, q_ps)
                        elif kt < 3:
                            emit_proj_cb(0, kt, y0)
                    rq = []
                    if p is not None:
                        rq = rot_ops(p, 0, q_ps)
                        rq[0]()   # psum evacuations go out immediately so
                        rq[1]()   # the shared qkv tile frees for k
                        rq = rq[2:]
                        k_ps = ps_qkv.tile([128, N], F32, tag="qkv", name=f"k{p}")
                    # phase B: lg(a,h1) | avt(a,h0) | qk-k(p) | rot-q thunks
                    if a >= 1:
                        emit_avt(a - 1, 1, 3)
                        emit_norm(a - 1, 1)
                        emit_transpose(a - 1)
                    for kt in range(8):
                        emit_lg_exp(a, 1, kt)
                        if kt in (0, 4):
                            emit_avt(a, 0, kt // 4)
                        if p is not None:
                            emit_qk_step(p, kt, 1, k_ps)
                        if rq:
                            rq.pop(0)()
                        if p is None and 3 <= kt < 6:
                            emit_proj_cb(0, kt, y0)
                    # phase C: v(p) region chains, padded with the two
                    # held-back avt chains while the k evacuation drains;
                    # rot-k thunks interleave with the chain evacuations
                    if p is not None:
                        rk = rot_ops(p, 1, k_ps)
                        rk[0]()
                        rk[1]()
                        rk = rk[2:]
                        v_ps = ps_qkv.tile([128, N], F32, tag="qkv", name=f"v{p}")
                        new_vext(p)
                        emit_avt(a, 0, 2)
                        if rk:
                            rk.pop(0)()
                        emit_avt(a, 0, 3)
                        if rk:
                            rk.pop(0)()
                        emit_v_chain(p, 0, v_ps, evac=False)
                        for tb in range(1, 8):
                            emit_v_chain(p, tb, v_ps, evac=(tb % 2 == 1))
                            if rk:
                                rk.pop(0)()
                        if p == 7:
                            # pre-issue slot 8's first logits+exp so the
                            # ACT-bound final slot starts with a head start
                            for kt in range(4):
                                emit_lg_exp(7, 0, kt)
                    else:
                        emit_avt(a, 0, 2)
                        emit_avt(a, 0, 3)
                    emit_norm(a, 0)
                    if DEBUG_TAPS and s == 1:
                        nc.sync.dma_start(out=dbg_e[:, :], in_=etiles[(0, 0, 0)])
                        nc.sync.dma_start(
                            out=dbg_ar[:, :],
                            in_=avraw[(0, 0)].rearrange("p a x -> p (a x)"),
                        )

                # ---- tail attention: pair 7 head 1 ----
                for j in range(4):
                    emit_avt(7, 1, j)
                emit_norm(7, 1)
                emit_transpose(7)
                if DEBUG_TAPS:
                    nc.sync.dma_start(
                        out=dbg_avn[:, :],
                        in_=avn_sb.rearrange("p a x -> p (a x)"),
                    )
                    nc.sync.dma_start(
                        out=dbg_at[:, :],
                        in_=attnT_sb.rearrange("p a x -> p (a x)"),
                    )

            # ---- proj tail (ps_lg/ps_av released -> 6 banks free).
            # cb=7 of every chain needs attnT(7), which lands ~2.5us after
            # the last avt; front-load cb 0..6 of the first chains so PE
            # never waits on the transpose.
            emit_proj_cb(0, 6, y0)
            with tc.tile_pool(name="ps_y", bufs=2, space="PSUM") as ps_y:
                ys = {0: y0}
                for qb in range(1, 3):
                    ys[qb] = ps_y.tile([128, C], F32, tag="yps", name=f"yps{qb}")
                    for cb in range(7):
                        emit_proj_cb(qb, cb, ys[qb])
                for qb in range(3):
                    emit_proj_cb(qb, 7, ys[qb])
                    emit_proj_bias(ys[qb])
                    finish_y(qb, ys[qb])
                for qb in range(3, 7):
                    y = ps_y.tile([128, C], F32, tag="yps", name=f"yps{qb}")
                    for cb in range(8):
                        emit_proj_cb(qb, cb, y)
                    emit_proj_bias(y)
                    finish_y(qb, y)
                # last block as two independent half-chains so the final
                # evacuate+DMA tail is half as deep
                yh = [
                    ps_y.tile([128, 512], F32, tag="yph", name=f"yph{hf}")
                    for hf in range(2)
                ]
                ysb7 = youtp.tile([128, C], BF16, tag="y", name="y7")
                for hf in range(2):
                    for cb in range(8):
                        nc.tensor.matmul(
                            yh[hf],
                            attnT_sb[:, cb, 7 * 128:8 * 128],
                            wp_sb[:, cb, hf * 512:(hf + 1) * 512],
                            start=(cb == 0),
                            stop=False,
                        )
                    nc.tensor.matmul(
                        yh[hf],
                        ones_sb,
                        brow_sb[:, hf * 512:(hf + 1) * 512],
                        start=False,
                        stop=True,
                    )
                    nc.scalar.copy(ysb7[:, hf * 512:(hf + 1) * 512], yh[hf])
                    nc.sync.dma_start(
                        out=out_h[7 * 128:8 * 128, hf * 512:(hf + 1) * 512],
                        in_=ysb7[:, hf * 512:(hf + 1) * 512],
                    )
    nc.finalize()
    return nc


_PROGRAM = None


def _host_weights(w_qkv):
    wq = w_qkv[:, 0:C]
    wk = w_qkv[:, C:2 * C]
    wv = w_qkv[:, 2 * C:3 * C]
    packs = []
    for p in range(NP):
        sl = slice(p * 128, (p + 1) * 128)
        packs.append(np.concatenate([wq[:, sl], wk[:, sl], wv[:, sl]], axis=1))
    return np.ascontiguousarray(
        np.concatenate(packs, axis=0)
    ).astype(NPBF16)  # (8192, 384)


def kernel(x, w_qkv, w_proj, b_proj):
    global _PROGRAM
    if _PROGRAM is None:
        _PROGRAM = _build_program()
    nc = _PROGRAM

    cos_d, sin_d = _host_tables()
    shared = {
        "wqkv": _host_weights(np.asarray(w_qkv, np.float32)),
        "w_proj": np.ascontiguousarray(w_proj).astype(NPBF16),
        "b_row": np.ascontiguousarray(b_proj, np.float32).reshape(1, C).astype(NPBF16),
        "cos_d": cos_d.astype(NPBF16),
        "sin_d": sin_d.astype(NPBF16),
        "ones_row": np.ones((1, 128), NPBF16),
    }
    in_maps = [
        {
            "xt": np.ascontiguousarray(np.asarray(x[b], np.float32).T).astype(NPBF16),
            **shared,
        }
        for b in range(NCORES)
    ]
    res = run_bass_kernel_spmd(nc, in_maps, core_ids=list(range(NCORES)))
    return np.stack(
        [res.results[b]["out"].astype(np.float32) for b in range(NCORES)], axis=0
    )


if __name__ == "__main__":
    xs = np.random.randn(B, N, C).astype(np.float32)
    wq = (np.random.randn(C, 3 * C) / np.sqrt(C)).astype(np.float32)
    wp = (np.random.randn(C, C) / np.sqrt(C)).astype(np.float32)
    bp = (np.random.randn(C) * 0.01).astype(np.float32)
    out = kernel(x=xs, w_qkv=wq, w_proj=wp, b_proj=bp)
    print(out.shape, out.dtype)
